# revision 9
# baseline (speedup 1.0000x reference)
"""ContrastiveLoss Trainium2 kernel (v2, fast path + exact fallback).

Math (matches the jax reference):
    an = l2norm(inputs_col); bn = l2norm(inputs_row)
    sim = an @ bn.T                                     [n, n]
    same = targets_col[:,None] == target_row[None,:]
    pos = same & (sim < 1-1e-5);  neg = ~same & (sim > 0.5)
    loss = sum(where(any(pos,1), sum(pos*(1-sim) + neg*sim, 1), 0)) / n

Fast path (this file's main kernel): for each [128,1024] sim block it
computes only
    negp  = sum_j same*(sim - c)        (c = 1-1e-5)        [DVE stt+accum]
    viol  = sum_j relu(sim - margin)                         [ACT relu+accum]
and gates rows by (-negp > 0.25). When viol == 0 this is EXACT:
    viol == 0  certifies  sim <= margin < c everywhere, hence
      *  neg mask empty and pos mask == same  ->  row_loss = sum same*(c-sim)
         (off from the reference's (1-sim) by 1e-5 per pos pair, ~1e-5 rel)
      *  -negp = sum same*(c-sim) >= 0.49 per matching pair, so
         (-negp > 0.25) == any(same) == has_pos.
The kernel returns (partial, viol). If viol > 0 on any core the host falls
back to the exact-for-all-inputs kernel (the v1 baseline, kept below).

Distribution: rows of inputs_col sharded across 8 cores (1024 each);
inputs_row / target_row replicated; host sums the 8 scalar partials.

Per-core fast pipeline (all compute on device; host only casts dtypes):
  prep (per quarter of 2048 B-rows, pipelined):
    1 batched DMA load (bf16), 16x ACT square+accum (row ssq),
    1 ACT sqrt + 1 DVE reciprocal (batched inv norms),
    16x Pool scale-cast to fp16, 16x single-instruction XBAR transpose
    ([128,1024] -> [128,8,128] k-tiled layout)
  main (per a-chunk x 1024-col block): 16 fp16 matmuls into PSUM,
    DVE is_equal mask, DVE stt accum (pos loss), ACT relu accum (viol).
"""

import numpy as np
from contextlib import ExitStack

import concourse.bass as bass
import concourse.mybir as mybir
import concourse.tile as tile
from concourse import bacc
from concourse.bass import ds, ts

N = 8192            # rows of inputs_col / inputs_row
D = 1024            # feature dim
NCORES = 8
ROWS = N // NCORES  # inputs_col rows per core
P = 128             # SBUF partitions
NCH = ROWS // P     # a-chunks per core (8)
KT = D // P         # contraction tiles (8)
QJ = 2048           # B-row quarter height
NQ = N // QJ        # quarters (4)
TQ = QJ // P        # tiles per quarter (16)
TG = 4              # tiles per prep group (staging granularity)
JB = 1024           # elementwise block width (2 PSUM banks)
NJB_Q = QJ // JB    # blocks per quarter (2)
NJB = N // JB       # blocks per full row (8)

EPS_NORM = 1e-12
EPS_POS = 1e-5
MARGIN = 0.5
CPOS = 1.0 - EPS_POS

F32 = mybir.dt.float32
F16 = mybir.dt.float16
BF16 = mybir.dt.bfloat16
AF = mybir.ActivationFunctionType
OP = mybir.AluOpType


def build_fast_body(tc, out_ap, a_ap, b_ap, tcol_ap, trow_ap):
    nc = tc.nc
    ctx = ExitStack()
    with ctx:
        singles = ctx.enter_context(tc.tile_pool(name="singles", bufs=1))
        small = ctx.enter_context(tc.tile_pool(name="small", bufs=4))
        junk = ctx.enter_context(tc.tile_pool(name="junk", bufs=3))
        stage = ctx.enter_context(tc.tile_pool(name="stage", bufs=3))
        xn_pool = ctx.enter_context(tc.tile_pool(name="xn", bufs=4))
        btq_pool = ctx.enter_context(tc.tile_pool(name="btq", bufs=2))
        ew_pool = ctx.enter_context(tc.tile_pool(name="ew", bufs=3))
        psum_mm = ctx.enter_context(
            tc.tile_pool(name="psum_mm", bufs=3, space=bass.MemorySpace.PSUM)
        )
        psum_fin = ctx.enter_context(
            tc.tile_pool(name="psum_fin", bufs=1, space=bass.MemorySpace.PSUM)
        )

        ones_col = singles.tile([P, 1], F32)
        nc.vector.memset(ones_col, 1.0)
        eps_tile = singles.tile([P, 1], F32)
        nc.vector.memset(eps_tile, EPS_NORM)
        mneg_tile = singles.tile([P, 1], F32)
        nc.vector.memset(mneg_tile, -MARGIN)

        # targets: issued on the ACT hwdge queue so the SP queue streams the
        # compute-critical loads back-to-back
        trow_bc = singles.tile([P, N], F16)
        trow_b = bass.AP(
            tensor=trow_ap.tensor,
            offset=trow_ap.offset,
            ap=[[0, P]] + list(trow_ap.ap),
        )
        nc.scalar.dma_start(out=trow_bc, in_=trow_b)
        tcol_sb = singles.tile([P, NCH], F32)
        nc.scalar.dma_start(
            out=tcol_sb, in_=tcol_ap.rearrange("(c p) -> p c", p=P)
        )

        # accumulation strips, one column per (chunk, jb) block
        negp_strip = singles.tile([P, NCH * NJB], F32)
        viol_strip = singles.tile([P, NCH * NJB], F32)

        def prep_tiles(src_ap, row0, ntiles, dst_bt, dst_col0):
            """Load ntiles*[128,D] rows, l2-normalize, fp16-cast, transpose
            into dst_bt[:, k, dst_col0 + t*128]. ntiles <= TG."""
            xs = stage.tile([P, TG, D], BF16, tag="ld")
            nc.sync.dma_start(
                out=xs[:, :ntiles],
                in_=src_ap[ds(row0, ntiles * P), :].rearrange(
                    "(t p) d -> p t d", p=P
                ),
            )
            ssq = small.tile([P, TG], F32, tag="ssq")
            for t in range(ntiles):
                sqj = junk.tile([P, D], BF16, tag="sqj")
                nc.scalar.activation(
                    sqj, xs[:, t], AF.Square, accum_out=ssq[:, t : t + 1]
                )
            nrm = small.tile([P, TG], F32, tag="nrm")
            nc.scalar.activation(
                nrm[:, :ntiles], ssq[:, :ntiles], AF.Sqrt, bias=eps_tile
            )
            inv = small.tile([P, TG], F32, tag="inv")
            nc.vector.reciprocal(inv[:, :ntiles], nrm[:, :ntiles])
            for t in range(ntiles):
                xn = xn_pool.tile([P, D], F16, tag="xn")
                nc.vector.tensor_scalar(
                    out=xn, in0=xs[:, t], scalar1=inv[:, t : t + 1],
                    scalar2=None, op0=OP.mult,
                )
                nc.sync.dma_start_transpose(
                    out=dst_bt[:, :, ds(dst_col0 + t * P, P)], in_=xn
                )

        # ---- A shard: normalize + transpose -> atn [128, KT, ROWS] fp16
        atn = singles.tile([P, KT, ROWS], F16)
        for g in range(NCH // TG):
            prep_tiles(a_ap, g * TG * P, TG, atn, g * TG * P)

        # ---- main loop over B quarters
        for q in range(NQ):
            btn = btq_pool.tile([P, KT, QJ], F16, tag="btn")
            for g in range(TQ // TG):
                prep_tiles(b_ap, q * QJ + g * TG * P, TG, btn, g * TG * P)

            for jb in range(NJB_Q):
                for c in range(NCH):
                    jbg = q * NJB_Q + jb
                    col = c * NJB + jbg
                    ps = psum_mm.tile([P, JB], F32, tag="ps")
                    for h in range(JB // 512):
                        for k in range(KT):
                            nc.tensor.matmul(
                                ps[:, ds(h * 512, 512)],
                                atn[:, k, ds(c * P, P)],
                                btn[:, k, ds(jb * JB + h * 512, 512)],
                                start=(k == 0),
                                stop=(k == KT - 1),
                            )
                    # s = (t_row == t_col[p])  [128, JB] fp16
                    s16 = ew_pool.tile([P, JB], F16, tag="s16")
                    nc.vector.tensor_scalar(
                        out=s16,
                        in0=trow_bc[:, ds(jbg * JB, JB)],
                        scalar1=tcol_sb[:, c : c + 1],
                        scalar2=None,
                        op0=OP.is_equal,
                    )
                    # negp += sum_j s * (sim - c)
                    j16 = junk.tile([P, JB], F16, tag="j16")
                    nc.vector.scalar_tensor_tensor(
                        out=j16,
                        in0=ps,
                        scalar=CPOS,
                        in1=s16,
                        op0=OP.subtract,
                        op1=OP.mult,
                        accum_out=negp_strip[:, col : col + 1],
                    )
                    # viol += sum_j relu(sim - margin)
                    v16 = ew_pool.tile([P, JB], F16, tag="v16")
                    nc.scalar.activation(
                        v16, ps, AF.Relu, bias=mneg_tile, scale=1.0,
                        accum_out=viol_strip[:, col : col + 1],
                    )

        # ---- finalize: row_loss = (-negp) * (-negp > 0.25); sum all rows
        loss_acc = singles.tile([P, 1], F32)
        nc.vector.memset(loss_acc, 0.0)
        viol_acc = singles.tile([P, 1], F32)
        nc.vector.memset(viol_acc, 0.0)
        for c in range(NCH):
            sl = ds(c * NJB, NJB)
            negp = small.tile([P, 1], F32, tag="negp")
            nc.vector.tensor_reduce(
                negp, negp_strip[:, sl], axis=mybir.AxisListType.X, op=OP.add
            )
            vio = small.tile([P, 1], F32, tag="vio")
            nc.vector.tensor_reduce(
                vio, viol_strip[:, sl], axis=mybir.AxisListType.X, op=OP.add
            )
            nc.vector.tensor_add(viol_acc, viol_acc, vio)
            # gated = (negp < -0.25) * negp  == -row_loss
            gated = small.tile([P, 1], F32, tag="gated")
            nc.vector.scalar_tensor_tensor(
                out=gated, in0=negp, scalar=-0.25, in1=negp,
                op0=OP.is_lt, op1=OP.mult,
            )
            nc.vector.tensor_sub(loss_acc, loss_acc, gated)

        pfin = psum_fin.tile([1, 2], F32)
        nc.tensor.matmul(pfin[:, 0:1], loss_acc, ones_col, start=True, stop=True)
        nc.tensor.matmul(pfin[:, 1:2], viol_acc, ones_col, start=True, stop=True)
        ob = small.tile([1, 2], F32, tag="ob")
        nc.vector.tensor_copy(ob, pfin)
        nc.sync.dma_start(out=out_ap, in_=ob)


_NC_CACHE = {}


def build_nc(reps=1):
    """Fast-path kernel. reps>1 wraps the body in a hardware For_i loop,
    used only for differential wall-clock timing."""
    key = ("fast", reps)
    if key in _NC_CACHE:
        return _NC_CACHE[key]
    nc = bacc.Bacc("TRN2", target_bir_lowering=False, debug=False)
    a_ap = nc.dram_tensor("a_shard", [ROWS, D], BF16, kind="ExternalInput").ap()
    b_ap = nc.dram_tensor("b_full", [N, D], BF16, kind="ExternalInput").ap()
    tcol_ap = nc.dram_tensor("t_col", [ROWS], F32, kind="ExternalInput").ap()
    trow_ap = nc.dram_tensor("t_row", [N], F16, kind="ExternalInput").ap()
    out_ap = nc.dram_tensor("pv", [1, 2], F32, kind="ExternalOutput").ap()
    with tile.TileContext(nc) as tc:
        if reps == 1:
            build_fast_body(tc, out_ap, a_ap, b_ap, tcol_ap, trow_ap)
        else:
            with tc.For_i(0, reps, 1):
                build_fast_body(tc, out_ap, a_ap, b_ap, tcol_ap, trow_ap)
    nc.compile()
    _NC_CACHE[key] = nc
    return nc


def make_in_maps(inputs_col, targets_col, inputs_row, target_row):
    import ml_dtypes

    b_full = np.ascontiguousarray(
        np.asarray(inputs_row, dtype=np.float32).astype(ml_dtypes.bfloat16)
    )
    trow = np.asarray(target_row).astype(np.float16)
    in_maps = []
    for c in range(NCORES):
        sl = slice(c * ROWS, (c + 1) * ROWS)
        in_maps.append(
            {
                "a_shard": np.ascontiguousarray(
                    np.asarray(inputs_col[sl], dtype=np.float32).astype(
                        ml_dtypes.bfloat16
                    )
                ),
                "b_full": b_full,
                "t_col": np.asarray(targets_col[sl]).astype(np.float32),
                "t_row": trow,
            }
        )
    return in_maps


# ---------------------------------------------------------------------------
# Exact fallback (v1 baseline): correct for ALL inputs, ~4x slower. Used only
# when the fast path reports viol > 0 (some sim > margin), which cannot happen
# for inputs whose cosine similarities stay below 0.5.
# ---------------------------------------------------------------------------


def _normalize_tile_exact(nc, pools, x_f32, x_bf, eps_tile):
    small, junk = pools
    P_ = 128
    sq = small.tile([P_, 1], F32, tag="sq")
    sqj = junk.tile([P_, D], BF16, tag="sqj")
    nc.scalar.activation(sqj, x_f32, AF.Square, accum_out=sq)
    nc.scalar.activation(sq, sq, AF.Sqrt, bias=eps_tile)
    inv = small.tile([P_, 1], F32, tag="inv")
    nc.vector.reciprocal(inv, sq)
    nc.scalar.activation(x_bf, x_f32, AF.Copy, bias=0.0, scale=inv)


def build_exact_body(tc, out_ap, a_ap, b_ap, tcol_ap, trow_ap):
    nc = tc.nc
    ctx = ExitStack()
    with ctx:
        from concourse.masks import make_identity

        singles = ctx.enter_context(tc.tile_pool(name="singles", bufs=1))
        small = ctx.enter_context(tc.tile_pool(name="small", bufs=6))
        junk = ctx.enter_context(tc.tile_pool(name="junk", bufs=4))
        stage_f32 = ctx.enter_context(tc.tile_pool(name="stage_f32", bufs=3))
        stage_bf = ctx.enter_context(tc.tile_pool(name="stage_bf", bufs=6))
        btq_pool = ctx.enter_context(tc.tile_pool(name="btq", bufs=2))
        ew_pool = ctx.enter_context(tc.tile_pool(name="ew", bufs=3))
        psum_mm = ctx.enter_context(
            tc.tile_pool(name="psum_mm", bufs=3, space=bass.MemorySpace.PSUM)
        )
        psum_fin = ctx.enter_context(
            tc.tile_pool(name="psum_fin", bufs=1, space=bass.MemorySpace.PSUM)
        )

        ident = singles.tile([P, P], BF16)
        make_identity(nc, ident)
        ones_col = singles.tile([P, 1], F32)
        nc.vector.memset(ones_col, 1.0)
        eps_tile = singles.tile([P, 1], F32)
        nc.vector.memset(eps_tile, EPS_NORM)

        trow_bc = singles.tile([P, N], F32)
        trow_b = bass.AP(
            tensor=trow_ap.tensor,
            offset=trow_ap.offset,
            ap=[[0, P]] + list(trow_ap.ap),
        )
        nc.sync.dma_start(out=trow_bc, in_=trow_b)

        tcol_sb = singles.tile([P, NCH], F32)
        tcol2 = tcol_ap.rearrange("(c p) -> c p", p=P)
        for c in range(NCH):
            nc.sync.dma_start(out=tcol_sb[:, c : c + 1], in_=tcol2[c][:, None])

        rq_strip = singles.tile([P, NCH * NJB], F32)
        rg_strip = singles.tile([P, NCH * NJB], F32)
        rsg_strip = singles.tile([P, NCH * NJB], F32)

        at_sb = singles.tile([P, KT, ROWS], BF16)
        for c in range(NCH):
            xf = stage_f32.tile([P, D], F32, tag="xf")
            nc.sync.dma_start(out=xf, in_=a_ap[ds(c * P, P), :])
            xb = stage_bf.tile([P, D], BF16, tag="xb")
            _normalize_tile_exact(nc, (small, junk), xf, xb, eps_tile)
            for k in range(KT):
                nc.sync.dma_start_transpose(
                    out=at_sb[:, k, ds(c * P, P)], in_=xb[:, ds(k * P, P)]
                )

        for q in range(NQ):
            bt = btq_pool.tile([P, KT, QJ], BF16, tag="bt")
            for t in range(QJ // P):
                row0 = q * QJ + t * P
                xf = stage_f32.tile([P, D], F32, tag="xf")
                nc.sync.dma_start(out=xf, in_=b_ap[ds(row0, P), :])
                xb = stage_bf.tile([P, D], BF16, tag="xb")
                _normalize_tile_exact(nc, (small, junk), xf, xb, eps_tile)
                for k in range(KT):
                    nc.sync.dma_start_transpose(
                        out=bt[:, k, ds(t * P, P)], in_=xb[:, ds(k * P, P)]
                    )

            for jb in range(NJB_Q):
                for c in range(NCH):
                    jbg = q * NJB_Q + jb
                    col = c * NJB + jbg
                    ps = psum_mm.tile([P, JB], F32, tag="ps")
                    for h in range(JB // 512):
                        for k in range(KT):
                            nc.tensor.matmul(
                                ps[:, ds(h * 512, 512)],
                                at_sb[:, k, ds(c * P, P)],
                                bt[:, k, ds(jb * JB + h * 512, 512)],
                                start=(k == 0),
                                stop=(k == KT - 1),
                            )
                    smb = ew_pool.tile([P, JB], BF16, tag="smb")
                    nc.scalar.activation(smb, ps, AF.Copy, bias=0.0, scale=1.0)
                    nfpos = ew_pool.tile([P, JB], BF16, tag="nfpos")
                    nc.gpsimd.tensor_scalar(
                        out=nfpos, in0=smb, scalar1=CPOS, scalar2=0.0,
                        op0=OP.subtract, op1=OP.min,
                    )
                    gsim = ew_pool.tile([P, JB], BF16, tag="gsim")
                    nc.vector.scalar_tensor_tensor(
                        out=gsim, in0=smb, scalar=MARGIN, in1=smb,
                        op0=OP.is_gt, op1=OP.mult,
                        accum_out=rg_strip[:, col : col + 1],
                    )
                    s = ew_pool.tile([P, JB], BF16, tag="s")
                    nc.gpsimd.tensor_scalar(
                        out=s, in0=trow_bc[:, ds(jbg * JB, JB)],
                        scalar1=tcol_sb[:, c : c + 1], scalar2=None,
                        op0=OP.is_equal,
                    )
                    j1 = junk.tile([P, JB], BF16, tag="j1")
                    nc.vector.scalar_tensor_tensor(
                        out=j1, in0=s, scalar=-1.0, in1=nfpos,
                        op0=OP.mult, op1=OP.mult,
                        accum_out=rq_strip[:, col : col + 1],
                    )
                    j2 = junk.tile([P, JB], BF16, tag="j2")
                    nc.vector.scalar_tensor_tensor(
                        out=j2, in0=s, scalar=1.0, in1=gsim,
                        op0=OP.mult, op1=OP.mult,
                        accum_out=rsg_strip[:, col : col + 1],
                    )

        loss_acc = singles.tile([P, 1], F32)
        nc.vector.memset(loss_acc, 0.0)
        for c in range(NCH):
            sl = ds(c * NJB, NJB)
            rq = small.tile([P, 1], F32, tag="rq")
            nc.vector.tensor_reduce(rq, rq_strip[:, sl], axis=mybir.AxisListType.X, op=OP.add)
            rg = small.tile([P, 1], F32, tag="rg")
            nc.vector.tensor_reduce(rg, rg_strip[:, sl], axis=mybir.AxisListType.X, op=OP.add)
            rsg = small.tile([P, 1], F32, tag="rsg")
            nc.vector.tensor_reduce(rsg, rsg_strip[:, sl], axis=mybir.AxisListType.X, op=OP.add)
            ind = small.tile([P, 1], F32, tag="ind")
            nc.vector.tensor_scalar(
                out=ind, in0=rq, scalar1=0.0, scalar2=None, op0=OP.is_gt
            )
            tmp = small.tile([P, 1], F32, tag="tmp")
            nc.vector.tensor_sub(tmp, rg, rsg)
            nc.vector.tensor_add(tmp, tmp, rq)
            nc.vector.tensor_mul(tmp, tmp, ind)
            nc.vector.tensor_add(loss_acc, loss_acc, tmp)

        pfin = psum_fin.tile([1, 1], F32)
        nc.tensor.matmul(pfin, loss_acc, ones_col, start=True, stop=True)
        ob = small.tile([1, 1], F32, tag="ob")
        nc.vector.tensor_copy(ob, pfin)
        nc.sync.dma_start(out=out_ap, in_=ob)


def build_nc_exact():
    key = ("exact", 1)
    if key in _NC_CACHE:
        return _NC_CACHE[key]
    nc = bacc.Bacc("TRN2", target_bir_lowering=False, debug=False)
    a_ap = nc.dram_tensor("a_shard", [ROWS, D], F32, kind="ExternalInput").ap()
    b_ap = nc.dram_tensor("b_full", [N, D], F32, kind="ExternalInput").ap()
    tcol_ap = nc.dram_tensor("t_col", [ROWS], F32, kind="ExternalInput").ap()
    trow_ap = nc.dram_tensor("t_row", [N], F32, kind="ExternalInput").ap()
    out_ap = nc.dram_tensor("partial", [1, 1], F32, kind="ExternalOutput").ap()
    with tile.TileContext(nc) as tc:
        build_exact_body(tc, out_ap, a_ap, b_ap, tcol_ap, trow_ap)
    nc.compile()
    _NC_CACHE[key] = nc
    return nc


def make_in_maps_exact(inputs_col, targets_col, inputs_row, target_row):
    b_full = np.ascontiguousarray(np.asarray(inputs_row, dtype=np.float32))
    trow = np.asarray(target_row).astype(np.float32)
    in_maps = []
    for c in range(NCORES):
        sl = slice(c * ROWS, (c + 1) * ROWS)
        in_maps.append(
            {
                "a_shard": np.ascontiguousarray(
                    np.asarray(inputs_col[sl], dtype=np.float32)
                ),
                "b_full": b_full,
                "t_col": np.asarray(targets_col[sl]).astype(np.float32),
                "t_row": trow,
            }
        )
    return in_maps


def kernel(**inputs):
    from concourse.bass_utils import run_bass_kernel_spmd

    args = (
        inputs["inputs_col"],
        inputs["targets_col"],
        inputs["inputs_row"],
        inputs["target_row"],
    )
    nc = build_nc()
    res = run_bass_kernel_spmd(
        nc, make_in_maps(*args), list(range(NCORES))
    ).results
    total = 0.0
    viol = 0.0
    for c in range(NCORES):
        total += float(res[c]["pv"][0, 0])
        viol += float(res[c]["pv"][0, 1])
    if viol > 1e-3:
        # some sim crossed the margin: fast-path shortcut invalid -> exact path
        nce = build_nc_exact()
        rese = run_bass_kernel_spmd(
            nce, make_in_maps_exact(*args), list(range(NCORES))
        ).results
        total = sum(float(rese[c]["partial"][0, 0]) for c in range(NCORES))
    return np.float32(total / N)
